# revision 8
# baseline (speedup 1.0000x reference)
# Dense-MoE (all experts active) Trainium2 kernel, expert-parallel over 8
# NeuronCores. Each core computes its expert's 2-layer MLP over all tokens:
#   fe_e = gelu(h @ W1[e] + b1[e]) @ (probs[e] * W2[e])
# then a chunked fp16 ReduceScatter(add) across the 8 cores sums the expert
# contributions; the host reassembles the full [B, D] output and adds the
# (token-independent) bias term sum_e probs[e]*b2[e].
#
# Layout: activations stay transposed on-chip; L2 output is [D, tokens].
#   hT   [IN, B]    fp16, pre-transposed on the host, fully SBUF-resident
#   hidT [H, blk]   = (W1 block).T @ hT per 128-row chunk, gelu+b1 via ACT
#   feT  [D, blk]   = (W2 block).T @ hidT accumulated over H chunks
#
# Structure notes (from HW trace analysis):
# - Under full 8-core load the PE pair period is ~263ns per 512-col fp16
#   matmul (chip-level clock throttle; single-core runs the same stream at
#   216ns). 2048 matmuls -> ~540us is the compute floor; everything else
#   here is about not adding to it:
# - W1 is repacked host-side so each L1 m-pass needs one contiguous 0.25MB
#   DMA; the first matmul can start after ~2.3MB of DMA instead of 8MB.
# - Token blocks of 1024 with PSUM bank pairs rotating through all 8 banks
#   keep the Gelu/drain engines off the PE's critical path.
# - ReduceScatter runs on the CC core with ~10us fixed latency per op, so
#   chunks are few and big (1MB), and the final chunk is token-split so the
#   last RS is small and queue-free. Input loads, fe drains, and out writes
#   go on different DMA queues to avoid head-of-line blocking.
import os
import sys

sys.path.insert(0, "/opt/trn_rl_repo")

import numpy as np

import concourse.mybir as mybir
from concourse import bacc, tile

B, E, IN, H, D = 4096, 8, 1024, 2048, 1024
NCORES = 8
P = 128
NBLK = 4                  # token blocks; phases L1(b) -> L2(b) run serially
BLK = B // NBLK           # 1024 tokens per block
NBANK = BLK // 512        # 2 PSUM banks per pass
KC1 = IN // P             # 8 contraction chunks, layer 1
MC1 = H // P              # 16 H chunks (layer-1 output rows)
DC2 = D // P              # 8 D chunks (layer-2 output rows)
NDG = 2                   # Dc groups per block (4 Dc = 512 D rows each)
GR = DC2 // NDG * P       # 512 D rows per RS chunk
RSR = GR // NCORES        # 64 rows each core receives per ReduceScatter

F32 = mybir.dt.float32

_CACHE = {}


def build(mm_dtype_name="float16", rs_dtype_name="float16"):
    mm_dt = getattr(mybir.dt, mm_dtype_name)
    rs_dt = getattr(mybir.dt, rs_dtype_name)
    assert mybir.dt.size(mm_dt) == 2, "matmul path requires a 16-bit dtype"
    nc = bacc.Bacc("TRN2", target_bir_lowering=False)

    hT = nc.declare_dram_parameter("ht", [IN, B], mm_dt, isOutput=False)
    # w1m[m*P + p, k*P + c] = W1[k*P + p, m*P + c]: one contiguous [P, IN]
    # row-slab per L1 m-pass, holding its 8 stationary k-tiles side by side.
    w1m = nc.declare_dram_parameter("w1m", [MC1 * P, IN], mm_dt,
                                    isOutput=False)
    b1t = nc.declare_dram_parameter("b1t", [P, MC1], F32, isOutput=False)
    w2 = nc.declare_dram_parameter("w2", [H, D], mm_dt, isOutput=False)
    # out rows: block-major, then Dc-group; cols: token within block
    out = nc.declare_dram_parameter("out", [NBLK * NDG * RSR, BLK], rs_dt,
                                    isOutput=True)

    with tile.TileContext(nc) as tc:
        with (
            tc.tile_pool(name="weights", bufs=1) as wpool,
            tc.tile_pool(name="consts", bufs=1) as cpool,
            tc.tile_pool(name="ht", bufs=1) as ht_pool,
            tc.tile_pool(name="hid", bufs=MC1) as hid_pool,
            tc.tile_pool(name="fe", bufs=2) as fe_pool,
            tc.tile_pool(name="ps", bufs=8, space="PSUM") as ps_pool,
            tc.tile_pool(name="dram", bufs=2, space="DRAM") as dram_pool,
        ):
            # --- DMAs, ordered so the first L1 pass starts ~7us in: block0's
            # hT slabs + the first two w1 m-slabs lead; everything else
            # (needed 10s..300s of us later) follows. All input loads go on
            # the sync queue; they are all dependency-free so no HOL risk.
            ht_sb = [[None] * KC1 for _ in range(NBLK)]
            for k in range(KC1):
                t_ = ht_pool.tile([P, BLK], mm_dt, tag=f"ht_0_{k}")
                nc.sync.dma_start(t_[:], hT[k * P:(k + 1) * P, 0:BLK])
                ht_sb[0][k] = t_
            w1_sb = []
            for m in range(MC1):
                t_ = wpool.tile([P, IN], mm_dt, tag=f"w1_{m}")
                nc.sync.dma_start(t_[:], w1m[m * P:(m + 1) * P, :])
                w1_sb.append(t_)
                if m == 1:
                    b1_sb = cpool.tile([P, MC1], F32, tag="b1")
                    nc.sync.dma_start(b1_sb[:], b1t[:])
                    for k in range(KC1):
                        t2 = ht_pool.tile([P, BLK], mm_dt, tag=f"ht_1_{k}")
                        nc.sync.dma_start(
                            t2[:], hT[k * P:(k + 1) * P, BLK:2 * BLK])
                        ht_sb[1][k] = t2
            w2_sb = []
            for hc in range(MC1):
                t_ = wpool.tile([P, D], mm_dt, tag=f"w2_{hc}")
                nc.sync.dma_start(t_[:], w2[hc * P:(hc + 1) * P, :])
                w2_sb.append(t_)
            for b in range(2, NBLK):
                for k in range(KC1):
                    t_ = ht_pool.tile([P, BLK], mm_dt, tag=f"ht_{b}_{k}")
                    nc.sync.dma_start(
                        t_[:], hT[k * P:(k + 1) * P, b * BLK:(b + 1) * BLK])
                    ht_sb[b][k] = t_

            for b in range(NBLK):
                # --- L1 phase: hidT[m] = gelu((W1 block m).T @ hT + b1[m]) ---
                hid_sb = []
                for m in range(MC1):
                    banks = [
                        ps_pool.tile([P, 512], F32, tag="ps", name=f"ps{j}")
                        for j in range(NBANK)
                    ]
                    for k in range(KC1):
                        for j in range(NBANK):
                            nc.tensor.matmul(
                                banks[j][:],
                                w1_sb[m][:, k * P:(k + 1) * P],
                                ht_sb[b][k][:, j * 512:(j + 1) * 512],
                                start=(k == 0),
                                stop=(k == KC1 - 1),
                            )
                    hm = hid_pool.tile([P, BLK], mm_dt, tag="hid")
                    for j in range(NBANK):
                        nc.scalar.activation(
                            hm[:, j * 512:(j + 1) * 512],
                            banks[j][:],
                            mybir.ActivationFunctionType.Gelu,
                            bias=b1_sb[:, m:m + 1],
                            scale=1.0,
                        )
                    hid_sb.append(hm)

                # --- L2 phase + grouped ReduceScatter ---
                # Dc groups of 4 -> RS input [512, BLK]. On the last block the
                # second group is token-split so the final RS is 0.25MB and
                # hits an idle CC queue.
                last = b == NBLK - 1
                for g in range(NDG):
                    fe_chunks = (
                        [dram_pool.tile([GR, 512], rs_dt, tag=f"feh{h_}",
                                        name=f"feh{h_}") for h_ in range(2)]
                        if (last and g == NDG - 1)
                        else [dram_pool.tile([GR, BLK], rs_dt, tag="fe_dram",
                                             name="fe_dram")]
                    )
                    for di in range(DC2 // NDG):
                        dc = g * (DC2 // NDG) + di
                        banks = [
                            ps_pool.tile([P, 512], F32, tag="ps",
                                         name=f"ps{j}")
                            for j in range(NBANK)
                        ]
                        for hc in range(MC1):
                            for j in range(NBANK):
                                nc.tensor.matmul(
                                    banks[j][:],
                                    w2_sb[hc][:, dc * P:(dc + 1) * P],
                                    hid_sb[hc][:, j * 512:(j + 1) * 512],
                                    start=(hc == 0),
                                    stop=(hc == MC1 - 1),
                                )
                        fe_sb = fe_pool.tile([P, BLK], rs_dt, tag="fe")
                        for j in range(NBANK):
                            nc.scalar.activation(
                                fe_sb[:, j * 512:(j + 1) * 512],
                                banks[j][:],
                                mybir.ActivationFunctionType.Copy,
                            )
                        if len(fe_chunks) == 1:
                            nc.scalar.dma_start(
                                fe_chunks[0][di * P:(di + 1) * P, :], fe_sb[:]
                            )
                        else:
                            for h_ in range(2):
                                nc.scalar.dma_start(
                                    fe_chunks[h_][di * P:(di + 1) * P, :],
                                    fe_sb[:, h_ * 512:(h_ + 1) * 512],
                                )
                    for h_, fc in enumerate(fe_chunks):
                        cols = BLK // len(fe_chunks)
                        rs_chunk = dram_pool.tile(
                            [RSR, cols], rs_dt, tag=f"rs{len(fe_chunks)}{h_}",
                            name="rs_chunk",
                        )
                        nc.gpsimd.collective_compute(
                            "ReduceScatter",
                            mybir.AluOpType.add,
                            replica_groups=[list(range(NCORES))],
                            ins=[fc[:]],
                            outs=[rs_chunk[:]],
                        )
                        nc.gpsimd.dma_start(
                            out[(b * NDG + g) * RSR:(b * NDG + g + 1) * RSR,
                                h_ * cols:(h_ + 1) * cols],
                            rs_chunk[:],
                        )

    nc.finalize()
    return nc


def _get_nc(mm_dtype_name, rs_dtype_name):
    key = (mm_dtype_name, rs_dtype_name)
    if key not in _CACHE:
        _CACHE[key] = build(mm_dtype_name, rs_dtype_name)
    return _CACHE[key]


def _run(inputs, mm_dtype_name="float16", trace=False):
    from concourse.bass_utils import run_bass_kernel_spmd

    import ml_dtypes

    rs_dtype_name = os.environ.get("MOE_RS_DTYPE", "float16")
    np_mm = {"bfloat16": ml_dtypes.bfloat16, "float16": np.float16}[
        mm_dtype_name
    ]
    h = np.ascontiguousarray(np.asarray(inputs["h"], dtype=np.float32))
    hT = np.ascontiguousarray(h.T.astype(np_mm))  # [IN, B]
    gate_logits = np.asarray(inputs["gate_logits"], dtype=np.float64)
    W1 = np.asarray(inputs["W1"], dtype=np.float32)
    b1 = np.asarray(inputs["b1"], dtype=np.float32)
    W2 = np.asarray(inputs["W2"], dtype=np.float32)
    b2 = np.asarray(inputs["b2"], dtype=np.float32)

    # gate: softmax over E (uniform for zero logits); fold into W2 per expert
    z = np.exp(gate_logits - gate_logits.max())
    probs = (z / z.sum()).astype(np.float32)

    in_maps = []
    for e in range(NCORES):
        w1e = W1[e].astype(np_mm)                      # [IN, H]
        # w1m[m*P + p, k*P + c] = W1[k*P + p, m*P + c]
        w1m = np.ascontiguousarray(
            w1e.reshape(KC1, P, MC1, P).transpose(2, 1, 0, 3)
            .reshape(MC1 * P, IN)
        )
        in_maps.append({
            "ht": hT,
            "w1m": w1m,
            "b1t": np.ascontiguousarray(b1[e].reshape(MC1, P).T),  # [P,MC1]
            "w2": np.ascontiguousarray((W2[e] * probs[e]).astype(np_mm)),
        })

    nc = _get_nc(mm_dtype_name, rs_dtype_name)
    res = run_bass_kernel_spmd(nc, in_maps, list(range(NCORES)), trace=trace)

    # Reassemble: core r's out row (b*NDG + g)*RSR + i at col t is global
    # D row g*GR + r*RSR + i, token b*BLK + t.
    feT = np.empty((D, B), dtype=np.float32)
    for r in range(NCORES):
        o = np.asarray(res.results[r]["out"], dtype=np.float32)
        for b in range(NBLK):
            for g in range(NDG):
                feT[g * GR + r * RSR: g * GR + (r + 1) * RSR,
                    b * BLK:(b + 1) * BLK] = (
                    o[(b * NDG + g) * RSR:(b * NDG + g + 1) * RSR, :]
                )
    final = feT.T.copy()
    final += (probs @ b2)[None, :]  # token-independent bias term
    return final, res


def kernel(**inputs):
    mm_dtype_name = os.environ.get("MOE_MM_DTYPE", "float16")
    final, _ = _run(inputs, mm_dtype_name=mm_dtype_name, trace=False)
    return final


# revision 10
# speedup vs baseline: 1.0233x; 1.0233x over previous
# Dense-MoE (all experts active) Trainium2 kernel, expert-parallel over 8
# NeuronCores. Each core computes its expert's 2-layer MLP over all tokens:
#   fe_e = gelu(h @ W1[e] + b1[e]) @ (probs[e] * W2[e])
# then a chunked fp16 ReduceScatter(add) across the 8 cores sums the expert
# contributions; the host reassembles the full [B, D] output and adds the
# (token-independent) bias term sum_e probs[e]*b2[e].
#
# Layout: activations stay transposed on-chip; L2 output is [D, tokens].
#   hT   [IN, B]    fp16, pre-transposed on the host, fully SBUF-resident
#   hidT [H, blk]   = (W1 block).T @ hT per 128-row chunk, gelu+b1 via ACT
#   feT  [D, blk]   = (W2 block).T @ hidT accumulated over H chunks
#
# Structure notes (from HW trace analysis):
# - Under full 8-core load the PE pair period is ~263ns per 512-col fp16
#   matmul (chip-level clock throttle; single-core runs the same stream at
#   216ns). 2048 matmuls -> ~540us is the compute floor; everything else
#   here is about not adding to it:
# - W1 is repacked host-side so each L1 m-pass needs one contiguous 0.25MB
#   DMA; the first matmul can start after ~2.3MB of DMA instead of 8MB.
# - Token blocks of 1024 with PSUM bank pairs rotating through all 8 banks
#   keep the Gelu/drain engines off the PE's critical path.
# - ReduceScatter runs on the CC core with ~10us fixed latency per op, so
#   chunks are few and big (1MB), and the final chunk is token-split so the
#   last RS is small and queue-free. Input loads, fe drains, and out writes
#   go on different DMA queues to avoid head-of-line blocking.
import os
import sys

sys.path.insert(0, "/opt/trn_rl_repo")

import numpy as np

import concourse.mybir as mybir
from concourse import bacc, tile

B, E, IN, H, D = 4096, 8, 1024, 2048, 1024
NCORES = 8
P = 128
NBLK = 4                  # token blocks; phases L1(b) -> L2(b) run serially
BLK = B // NBLK           # 1024 tokens per block
NBANK = BLK // 512        # 2 PSUM banks per pass
KC1 = IN // P             # 8 contraction chunks, layer 1
MC1 = H // P              # 16 H chunks (layer-1 output rows)
DC2 = D // P              # 8 D chunks (layer-2 output rows)
# ReduceScatter chunk schedule: (block, dc_lo, n_dc). Early blocks ship one
# big chunk (RS overhead is ~9us/op); the last block ships Dc-pairs so the
# final RS is small and the CC queue is empty when it arrives.
CHUNKS = [
    (0, 0, 8),
    (1, 0, 8),
    (2, 0, 4), (2, 4, 4),
    (3, 0, 2), (3, 2, 2), (3, 4, 2), (3, 6, 2),
]
OUT_OFF = {}
_off = 0
for _b, _lo, _n in CHUNKS:
    OUT_OFF[(_b, _lo)] = _off
    _off += _n * P // NCORES
OUT_ROWS = _off

F32 = mybir.dt.float32

_CACHE = {}


def build(mm_dtype_name="float16", rs_dtype_name="float16"):
    mm_dt = getattr(mybir.dt, mm_dtype_name)
    rs_dt = getattr(mybir.dt, rs_dtype_name)
    assert mybir.dt.size(mm_dt) == 2, "matmul path requires a 16-bit dtype"
    nc = bacc.Bacc("TRN2", target_bir_lowering=False)

    # htp[p, b*8192 + k*1024 + t] = h[b*1024 + t, k*128 + p]: each block's
    # hT slabs are one contiguous [128, 8192] region -> one DMA per block
    # (DMA issue costs ~650ns of sequencer time each, so fewer is faster).
    htp = nc.declare_dram_parameter("htp", [P, NBLK * KC1 * BLK], mm_dt,
                                    isOutput=False)
    # w1m[m*P + p, k*P + c] = W1[k*P + p, m*P + c]: one contiguous [P, IN]
    # row-slab per L1 m-pass, holding its 8 stationary k-tiles side by side.
    w1m = nc.declare_dram_parameter("w1m", [MC1 * P, IN], mm_dt,
                                    isOutput=False)
    b1t = nc.declare_dram_parameter("b1t", [P, MC1], F32, isOutput=False)
    w2 = nc.declare_dram_parameter("w2", [H, D], mm_dt, isOutput=False)
    # out rows: block-major, then Dc-group; cols: token within block
    out = nc.declare_dram_parameter("out", [OUT_ROWS, BLK], rs_dt,
                                    isOutput=True)

    with tile.TileContext(nc) as tc:
        with (
            tc.tile_pool(name="weights", bufs=1) as wpool,
            tc.tile_pool(name="consts", bufs=1) as cpool,
            tc.tile_pool(name="ht", bufs=1) as ht_pool,
            tc.tile_pool(name="hid", bufs=MC1) as hid_pool,
            tc.tile_pool(name="fe", bufs=2) as fe_pool,
            tc.tile_pool(name="ps", bufs=8, space="PSUM") as ps_pool,
            tc.tile_pool(name="dram", bufs=2, space="DRAM") as dram_pool,
        ):
            # --- DMAs, ordered so the first L1 pass starts ~7us in: block0's
            # hT slabs + the first two w1 m-slabs lead; everything else
            # (needed 10s..300s of us later) follows. All input loads go on
            # the sync queue; they are all dependency-free so no HOL risk.
            # block0 as two half DMAs (k0-3, k4-7) so the first chains can
            # start ~3us earlier; blocks 1-3 one DMA each.
            ht_tiles = {}
            W = KC1 * BLK  # 8192 cols per block
            h0a = ht_pool.tile([P, W // 2], mm_dt, tag="ht_0a")
            nc.sync.dma_start(h0a[:], htp[:, 0:W // 2])
            h0b = ht_pool.tile([P, W // 2], mm_dt, tag="ht_0b")
            nc.sync.dma_start(h0b[:], htp[:, W // 2:W])

            def ht_slab(b, k):
                if b == 0:
                    t_ = h0a if k < KC1 // 2 else h0b
                    kk = k % (KC1 // 2)
                    return t_[:, kk * BLK:(kk + 1) * BLK]
                t_ = ht_tiles[b]
                return t_[:, k * BLK:(k + 1) * BLK]

            w1_sb = []
            for m in range(MC1):
                t_ = wpool.tile([P, IN], mm_dt, tag=f"w1_{m}")
                nc.sync.dma_start(t_[:], w1m[m * P:(m + 1) * P, :])
                w1_sb.append(t_)
                if m == 1:
                    b1_sb = cpool.tile([P, MC1], F32, tag="b1")
                    nc.sync.dma_start(b1_sb[:], b1t[:])
                    t2 = ht_pool.tile([P, W], mm_dt, tag="ht_1")
                    nc.sync.dma_start(t2[:], htp[:, W:2 * W])
                    ht_tiles[1] = t2
            w2_sb = []
            for hc in range(MC1):
                t_ = wpool.tile([P, D], mm_dt, tag=f"w2_{hc}")
                nc.sync.dma_start(t_[:], w2[hc * P:(hc + 1) * P, :])
                w2_sb.append(t_)
            for b in range(2, NBLK):
                t_ = ht_pool.tile([P, W], mm_dt, tag=f"ht_{b}")
                nc.sync.dma_start(t_[:], htp[:, b * W:(b + 1) * W])
                ht_tiles[b] = t_

            for b in range(NBLK):
                # --- L1 phase: hidT[m] = gelu((W1 block m).T @ hT + b1[m]) ---
                hid_sb = []
                for m in range(MC1):
                    banks = [
                        ps_pool.tile([P, 512], F32, tag="ps", name=f"ps{j}")
                        for j in range(NBANK)
                    ]
                    for k in range(KC1):
                        for j in range(NBANK):
                            nc.tensor.matmul(
                                banks[j][:],
                                w1_sb[m][:, k * P:(k + 1) * P],
                                ht_slab(b, k)[:, j * 512:(j + 1) * 512],
                                start=(k == 0),
                                stop=(k == KC1 - 1),
                            )
                    hm = hid_pool.tile([P, BLK], mm_dt, tag="hid")
                    for j in range(NBANK):
                        nc.scalar.activation(
                            hm[:, j * 512:(j + 1) * 512],
                            banks[j][:],
                            mybir.ActivationFunctionType.Gelu,
                            bias=b1_sb[:, m:m + 1],
                            scale=1.0,
                        )
                    hid_sb.append(hm)

                # --- L2 phase + chunked ReduceScatter ---
                # CC-core RS wall time ~ 9us fixed + payload*(7/8)/~70GB/s,
                # so chunks shrink over the run: 2MB while CC has slack in
                # the L1 windows, 0.5MB Dc-pairs at the end so the final RS
                # is short and starts on an idle CC queue.
                chunks = [c for c in CHUNKS if c[0] == b]
                ci = 0
                fe_chunk = None
                for dc in range(DC2):
                    blk_, dc_lo, dc_n = chunks[ci]
                    if dc == dc_lo:
                        fe_chunk = dram_pool.tile(
                            [dc_n * P, BLK], rs_dt, tag=f"fe{dc_n}",
                            name="fe_chunk", bufs=2,
                        )
                    banks = [
                        ps_pool.tile([P, 512], F32, tag="ps", name=f"ps{j}")
                        for j in range(NBANK)
                    ]
                    for hc in range(MC1):
                        for j in range(NBANK):
                            nc.tensor.matmul(
                                banks[j][:],
                                w2_sb[hc][:, dc * P:(dc + 1) * P],
                                hid_sb[hc][:, j * 512:(j + 1) * 512],
                                start=(hc == 0),
                                stop=(hc == MC1 - 1),
                            )
                    fe_sb = fe_pool.tile([P, BLK], rs_dt, tag="fe")
                    for j in range(NBANK):
                        nc.scalar.activation(
                            fe_sb[:, j * 512:(j + 1) * 512],
                            banks[j][:],
                            mybir.ActivationFunctionType.Copy,
                        )
                    nc.scalar.dma_start(
                        fe_chunk[(dc - dc_lo) * P:(dc - dc_lo + 1) * P, :],
                        fe_sb[:],
                    )
                    if dc == dc_lo + dc_n - 1:
                        rs_chunk = dram_pool.tile(
                            [dc_n * P // NCORES, BLK], rs_dt,
                            tag=f"rs{dc_n}", name="rs_chunk", bufs=2,
                        )
                        nc.gpsimd.collective_compute(
                            "ReduceScatter",
                            mybir.AluOpType.add,
                            replica_groups=[list(range(NCORES))],
                            ins=[fe_chunk[:]],
                            outs=[rs_chunk[:]],
                        )
                        nc.gpsimd.dma_start(
                            out[OUT_OFF[(b, dc_lo)]:
                                OUT_OFF[(b, dc_lo)] + dc_n * P // NCORES, :],
                            rs_chunk[:],
                        )
                        ci += 1

    nc.finalize()
    return nc


def _get_nc(mm_dtype_name, rs_dtype_name):
    key = (mm_dtype_name, rs_dtype_name)
    if key not in _CACHE:
        _CACHE[key] = build(mm_dtype_name, rs_dtype_name)
    return _CACHE[key]


def _run(inputs, mm_dtype_name="float16", trace=False):
    from concourse.bass_utils import run_bass_kernel_spmd

    import ml_dtypes

    rs_dtype_name = os.environ.get("MOE_RS_DTYPE", "float16")
    np_mm = {"bfloat16": ml_dtypes.bfloat16, "float16": np.float16}[
        mm_dtype_name
    ]
    h = np.ascontiguousarray(np.asarray(inputs["h"], dtype=np.float32))
    hT = h.T.astype(np_mm)  # [IN, B]
    htp = np.ascontiguousarray(
        hT.reshape(KC1, P, NBLK, BLK).transpose(1, 2, 0, 3)
        .reshape(P, NBLK * KC1 * BLK)
    )
    gate_logits = np.asarray(inputs["gate_logits"], dtype=np.float64)
    W1 = np.asarray(inputs["W1"], dtype=np.float32)
    b1 = np.asarray(inputs["b1"], dtype=np.float32)
    W2 = np.asarray(inputs["W2"], dtype=np.float32)
    b2 = np.asarray(inputs["b2"], dtype=np.float32)

    # gate: softmax over E (uniform for zero logits); fold into W2 per expert
    z = np.exp(gate_logits - gate_logits.max())
    probs = (z / z.sum()).astype(np.float32)

    in_maps = []
    for e in range(NCORES):
        w1e = W1[e].astype(np_mm)                      # [IN, H]
        # w1m[m*P + p, k*P + c] = W1[k*P + p, m*P + c]
        w1m = np.ascontiguousarray(
            w1e.reshape(KC1, P, MC1, P).transpose(2, 1, 0, 3)
            .reshape(MC1 * P, IN)
        )
        in_maps.append({
            "htp": htp,
            "w1m": w1m,
            "b1t": np.ascontiguousarray(b1[e].reshape(MC1, P).T),  # [P,MC1]
            "w2": np.ascontiguousarray((W2[e] * probs[e]).astype(np_mm)),
        })

    nc = _get_nc(mm_dtype_name, rs_dtype_name)
    res = run_bass_kernel_spmd(nc, in_maps, list(range(NCORES)), trace=trace)

    # Reassemble: chunk (b, dc_lo, n): core r's rows OUT_OFF..+n*16 are
    # global D rows dc_lo*128 + r*n*16 + i for tokens of block b.
    feT = np.empty((D, B), dtype=np.float32)
    for r in range(NCORES):
        o = np.asarray(res.results[r]["out"], dtype=np.float32)
        for (b, dc_lo, n) in CHUNKS:
            rpc = n * P // NCORES
            off = OUT_OFF[(b, dc_lo)]
            feT[dc_lo * P + r * rpc: dc_lo * P + (r + 1) * rpc,
                b * BLK:(b + 1) * BLK] = o[off:off + rpc, :]
    final = feT.T.copy()
    final += (probs @ b2)[None, :]  # token-independent bias term
    return final, res


def kernel(**inputs):
    mm_dtype_name = os.environ.get("MOE_MM_DTYPE", "float16")
    final, _ = _run(inputs, mm_dtype_name=mm_dtype_name, trace=False)
    return final


# revision 11
# speedup vs baseline: 1.0326x; 1.0091x over previous
# Dense-MoE (all experts active) Trainium2 kernel, expert-parallel over 8
# NeuronCores. Each core computes its expert's 2-layer MLP over all tokens:
#   fe_e = gelu(h @ W1[e] + b1[e]) @ (probs[e] * W2[e])
# then a chunked fp16 ReduceScatter(add) across the 8 cores sums the expert
# contributions; the host reassembles the full [B, D] output and adds the
# (token-independent) bias term sum_e probs[e]*b2[e].
#
# Layout: activations stay transposed on-chip; L2 output is [D, tokens].
#   hT   [IN, B]    fp16, pre-transposed on the host, fully SBUF-resident
#   hidT [H, blk]   = (W1 block).T @ hT per 128-row chunk, gelu+b1 via ACT
#   feT  [D, blk]   = (W2 block).T @ hidT accumulated over H chunks
#
# Structure notes (from HW trace analysis):
# - Under full 8-core load the PE pair period is ~263ns per 512-col fp16
#   matmul (chip-level clock throttle; a single core runs the same stream at
#   216ns). 2048 matmuls -> ~537us is the compute floor; everything else
#   here is about not adding to it.
# - W1 is repacked host-side so each L1 m-pass needs one contiguous 0.25MB
#   DMA, and hT is packed so each token block is one contiguous DMA (each
#   dma_start costs ~650ns of sequencer issue time). The first block's hT
#   arrives in 0.5MB quarters so the first chains start ~10us in.
# - PSUM bank groups rotate through all 8 banks so the Gelu/drain engines
#   never gate the next pass's matmuls.
# - ReduceScatter wall time is ~10us fixed + payload/(~60GB/s) on the one
#   CC core, so blocks shrink over the run (1024,1024,1536,512 tokens):
#   early blocks ship big chunks during ample compute windows; the final
#   512-token block ships four 0.25MB Dc-pair chunks so the last RS is
#   short and hits an idle CC queue. fe drains ride the scalar queue and
#   out writes the gpsimd queue to avoid head-of-line blocking on loads.
import os
import sys

sys.path.insert(0, "/opt/trn_rl_repo")

import numpy as np

import concourse.mybir as mybir
from concourse import bacc, tile

B, E, IN, H, D = 4096, 8, 1024, 2048, 1024
NCORES = 8
P = 128
KC1 = IN // P             # 8 contraction chunks, layer 1
MC1 = H // P              # 16 H chunks (layer-1 output rows)
DC2 = D // P              # 8 D chunks (layer-2 output rows)

BLOCKS = [1024, 1024, 1536, 512]          # tokens per phase-block
TOFF = [sum(BLOCKS[:i]) for i in range(len(BLOCKS) + 1)]
NBLK = len(BLOCKS)
MAXBLK = max(BLOCKS)

# ReduceScatter chunk schedule: (block, dc_lo, n_dc).
CHUNKS = [
    (0, 0, 8),
    (1, 0, 8),
    (2, 0, 4), (2, 4, 4),
    (3, 0, 2), (3, 2, 2), (3, 4, 2), (3, 6, 2),
]

F32 = mybir.dt.float32

_CACHE = {}


def build(mm_dtype_name="float16", rs_dtype_name="float16"):
    mm_dt = getattr(mybir.dt, mm_dtype_name)
    rs_dt = getattr(mybir.dt, rs_dtype_name)
    assert mybir.dt.size(mm_dt) == 2, "matmul path requires a 16-bit dtype"
    nc = bacc.Bacc("TRN2", target_bir_lowering=False)

    # htp: per-block contiguous segments; within block b (BLK tokens):
    # htp[p, TOFF[b]*KC1 + k*BLK + t] = h[TOFF[b] + t, k*P + p]
    htp = nc.declare_dram_parameter("htp", [P, KC1 * B], mm_dt,
                                    isOutput=False)
    # w1m[m*P + p, k*P + c] = W1[k*P + p, m*P + c]
    w1m = nc.declare_dram_parameter("w1m", [MC1 * P, IN], mm_dt,
                                    isOutput=False)
    b1t = nc.declare_dram_parameter("b1t", [P, MC1], F32, isOutput=False)
    w2 = nc.declare_dram_parameter("w2", [H, D], mm_dt, isOutput=False)
    # one output param per distinct chunk column count
    out_rows = {}
    for (b, dc_lo, n) in CHUNKS:
        cols = BLOCKS[b]
        out_rows[cols] = out_rows.get(cols, 0) + n * P // NCORES
    outs = {
        cols: nc.declare_dram_parameter(f"out{cols}", [rows, cols], rs_dt,
                                        isOutput=True)
        for cols, rows in out_rows.items()
    }
    out_off = {}
    _pos = {cols: 0 for cols in out_rows}
    for (b, dc_lo, n) in CHUNKS:
        cols = BLOCKS[b]
        out_off[(b, dc_lo)] = _pos[cols]
        _pos[cols] += n * P // NCORES

    with tile.TileContext(nc) as tc:
        with (
            tc.tile_pool(name="weights", bufs=1) as wpool,
            tc.tile_pool(name="consts", bufs=1) as cpool,
            tc.tile_pool(name="ht", bufs=1) as ht_pool,
            tc.tile_pool(name="hid", bufs=MC1) as hid_pool,
            tc.tile_pool(name="fe", bufs=2) as fe_pool,
            tc.tile_pool(name="ps", bufs=8, space="PSUM") as ps_pool,
            tc.tile_pool(name="dram", bufs=2, space="DRAM") as dram_pool,
        ):
            # --- input DMAs, ordered for the earliest possible L1 start ---
            ht_tiles = {}
            h0q = []
            for q in range(4):  # block0 in 2-slab quarters
                t_ = ht_pool.tile([P, 2 * BLOCKS[0]], mm_dt, tag=f"ht0_{q}")
                nc.sync.dma_start(
                    t_[:],
                    htp[:, q * 2 * BLOCKS[0]:(q + 1) * 2 * BLOCKS[0]],
                )
                h0q.append(t_)

            def ht_slab(b, k):
                if b == 0:
                    return h0q[k // 2][:, (k % 2) * BLOCKS[0]:
                                      (k % 2 + 1) * BLOCKS[0]]
                t_ = ht_tiles[b]
                return t_[:, k * BLOCKS[b]:(k + 1) * BLOCKS[b]]

            w1_sb = []
            for m in range(MC1):
                t_ = wpool.tile([P, IN], mm_dt, tag=f"w1_{m}")
                nc.sync.dma_start(t_[:], w1m[m * P:(m + 1) * P, :])
                w1_sb.append(t_)
                if m == 1:
                    b1_sb = cpool.tile([P, MC1], F32, tag="b1")
                    nc.sync.dma_start(b1_sb[:], b1t[:])
                    t2 = ht_pool.tile([P, KC1 * BLOCKS[1]], mm_dt, tag="ht_1")
                    nc.sync.dma_start(
                        t2[:],
                        htp[:, TOFF[1] * KC1:TOFF[2] * KC1],
                    )
                    ht_tiles[1] = t2
            w2_sb = []
            for hc in range(MC1):
                t_ = wpool.tile([P, D], mm_dt, tag=f"w2_{hc}")
                nc.sync.dma_start(t_[:], w2[hc * P:(hc + 1) * P, :])
                w2_sb.append(t_)
            for b in range(2, NBLK):
                t_ = ht_pool.tile([P, KC1 * BLOCKS[b]], mm_dt, tag=f"ht_{b}")
                nc.sync.dma_start(
                    t_[:], htp[:, TOFF[b] * KC1:TOFF[b + 1] * KC1]
                )
                ht_tiles[b] = t_

            for b in range(NBLK):
                blk = BLOCKS[b]
                nbank = blk // 512
                # --- L1: hidT[m] = gelu((W1 block m).T @ hT + b1[m]) ---
                hid_sb = []
                for m in range(MC1):
                    banks = [
                        ps_pool.tile([P, 512], F32, tag="ps", name=f"ps{j}")
                        for j in range(nbank)
                    ]
                    for k in range(KC1):
                        for j in range(nbank):
                            nc.tensor.matmul(
                                banks[j][:],
                                w1_sb[m][:, k * P:(k + 1) * P],
                                ht_slab(b, k)[:, j * 512:(j + 1) * 512],
                                start=(k == 0),
                                stop=(k == KC1 - 1),
                            )
                    hm = hid_pool.tile([P, MAXBLK], mm_dt, tag="hid")
                    for j in range(nbank):
                        nc.scalar.activation(
                            hm[:, j * 512:(j + 1) * 512],
                            banks[j][:],
                            mybir.ActivationFunctionType.Gelu,
                            bias=b1_sb[:, m:m + 1],
                            scale=1.0,
                        )
                    hid_sb.append(hm)

                # --- L2 + chunked ReduceScatter per the schedule ---
                chunks = [c for c in CHUNKS if c[0] == b]
                ci = 0
                fe_chunk = None
                for dc in range(DC2):
                    blk_, dc_lo, dc_n = chunks[ci]
                    if dc == dc_lo:
                        fe_chunk = dram_pool.tile(
                            [dc_n * P, blk], rs_dt, tag=f"fe{dc_n}_{blk}",
                            name="fe_chunk", bufs=2,
                        )
                    banks = [
                        ps_pool.tile([P, 512], F32, tag="ps", name=f"ps{j}")
                        for j in range(nbank)
                    ]
                    for hc in range(MC1):
                        for j in range(nbank):
                            nc.tensor.matmul(
                                banks[j][:],
                                w2_sb[hc][:, dc * P:(dc + 1) * P],
                                hid_sb[hc][:, j * 512:(j + 1) * 512],
                                start=(hc == 0),
                                stop=(hc == MC1 - 1),
                            )
                    fe_sb = fe_pool.tile([P, MAXBLK], rs_dt, tag="fe")
                    for j in range(nbank):
                        nc.scalar.activation(
                            fe_sb[:, j * 512:(j + 1) * 512],
                            banks[j][:],
                            mybir.ActivationFunctionType.Copy,
                        )
                    nc.scalar.dma_start(
                        fe_chunk[(dc - dc_lo) * P:(dc - dc_lo + 1) * P, :],
                        fe_sb[:, 0:blk],
                    )
                    if dc == dc_lo + dc_n - 1:
                        rs_chunk = dram_pool.tile(
                            [dc_n * P // NCORES, blk], rs_dt,
                            tag=f"rs{dc_n}_{blk}", name="rs_chunk", bufs=2,
                        )
                        nc.gpsimd.collective_compute(
                            "ReduceScatter",
                            mybir.AluOpType.add,
                            replica_groups=[list(range(NCORES))],
                            ins=[fe_chunk[:]],
                            outs=[rs_chunk[:]],
                        )
                        off = out_off[(b, dc_lo)]
                        nc.gpsimd.dma_start(
                            outs[blk][off:off + dc_n * P // NCORES, :],
                            rs_chunk[:],
                        )
                        ci += 1

    nc.finalize()
    return nc


def _get_nc(mm_dtype_name, rs_dtype_name):
    key = (mm_dtype_name, rs_dtype_name)
    if key not in _CACHE:
        _CACHE[key] = build(mm_dtype_name, rs_dtype_name)
    return _CACHE[key]


def _run(inputs, mm_dtype_name="float16", trace=False):
    from concourse.bass_utils import run_bass_kernel_spmd

    import ml_dtypes

    rs_dtype_name = os.environ.get("MOE_RS_DTYPE", "float16")
    np_mm = {"bfloat16": ml_dtypes.bfloat16, "float16": np.float16}[
        mm_dtype_name
    ]
    h = np.ascontiguousarray(np.asarray(inputs["h"], dtype=np.float32))
    hT = h.T.astype(np_mm)  # [IN, B]
    htp = np.concatenate(
        [
            np.ascontiguousarray(
                hT[:, TOFF[b]:TOFF[b + 1]]
                .reshape(KC1, P, BLOCKS[b]).transpose(1, 0, 2)
                .reshape(P, KC1 * BLOCKS[b])
            )
            for b in range(NBLK)
        ],
        axis=1,
    )
    gate_logits = np.asarray(inputs["gate_logits"], dtype=np.float64)
    W1 = np.asarray(inputs["W1"], dtype=np.float32)
    b1 = np.asarray(inputs["b1"], dtype=np.float32)
    W2 = np.asarray(inputs["W2"], dtype=np.float32)
    b2 = np.asarray(inputs["b2"], dtype=np.float32)

    # gate: softmax over E (uniform for zero logits); fold into W2 per expert
    z = np.exp(gate_logits - gate_logits.max())
    probs = (z / z.sum()).astype(np.float32)

    in_maps = []
    for e in range(NCORES):
        w1e = W1[e].astype(np_mm)                      # [IN, H]
        w1m = np.ascontiguousarray(
            w1e.reshape(KC1, P, MC1, P).transpose(2, 1, 0, 3)
            .reshape(MC1 * P, IN)
        )
        in_maps.append({
            "htp": htp,
            "w1m": w1m,
            "b1t": np.ascontiguousarray(b1[e].reshape(MC1, P).T),  # [P,MC1]
            "w2": np.ascontiguousarray((W2[e] * probs[e]).astype(np_mm)),
        })

    nc = _get_nc(mm_dtype_name, rs_dtype_name)
    res = run_bass_kernel_spmd(nc, in_maps, list(range(NCORES)), trace=trace)

    # Reassemble. Chunk (b, dc_lo, n): core r's rows in out<cols> are global
    # D rows dc_lo*128 + r*(n*16) + i, tokens TOFF[b]..TOFF[b+1].
    out_pos = {}
    feT = np.empty((D, B), dtype=np.float32)
    for r in range(NCORES):
        o_by_cols = {}
        pos = {}
        for (b, dc_lo, n) in CHUNKS:
            cols = BLOCKS[b]
            if cols not in o_by_cols:
                o_by_cols[cols] = np.asarray(
                    res.results[r][f"out{cols}"], dtype=np.float32
                )
                pos[cols] = 0
            rpc = n * P // NCORES
            o = o_by_cols[cols]
            feT[dc_lo * P + r * rpc: dc_lo * P + (r + 1) * rpc,
                TOFF[b]:TOFF[b + 1]] = o[pos[cols]:pos[cols] + rpc, :]
            pos[cols] += rpc
    final = feT.T.copy()
    final += (probs @ b2)[None, :]  # token-independent bias term
    return final, res


def kernel(**inputs):
    mm_dtype_name = os.environ.get("MOE_MM_DTYPE", "float16")
    final, _ = _run(inputs, mm_dtype_name=mm_dtype_name, trace=False)
    return final
